# revision 1
# baseline (speedup 1.0000x reference)
"""DFL loss (nn_DFLLoss) Trainium2 Bass kernel — 8-core data parallel.

reference computes, per (batch, pixel, coord j in 0..3):
    rl[b, hw, j, k] = reg_logits[b, j*8+k, hw]          (k in 0..7 bins)
    t = clip(targets, 0, 6.9999); l = floor(t); u = l+1
    per = w_l * (lse - rl[l]) + w_u * (lse - rl[u]),  lse = logsumexp_k rl
    loss = sum(per * pos_mask) / (max(sum(pos_mask), 1) * 4)

Key identity used here (removes the gather):
    w_l*rl[l] + w_u*rl[u] = sum_k relu(1 - |t - k|) * rl[k]
so masked_total = sum(mask*lse) - sum_k relu(1-|t-k|)*rl[k]*mask. The
hat-product+reduce runs as ONE fused custom DVE op per (batch, coord)
with the bin index k supplied by PageIdx over the 8 channel pages.
The mask is folded into t'' = clip(t) + 100*mask and the op evaluates
relu(1 - |t'' - (100 + k)|): positive pixels give |t - k|, masked-out
pixels give |t - 100 - k| >= 93 so every hat weight is exactly 0.

Per-core layout (4 batches, processed as 4 pipeline phases): partition
p = pixel-block (HW = 25600 = 128 blocks x 200 px). Channels live in the
free dimension, so Sum_k exp(rl) is a 3-level pairwise tree of bf16
tensor_tensor adds (2x mode), exp/ln run full-width on ScalarE, and the
small masked accumulations run on GpSimd (which never contends with the
1-port DVE op mix used here).
"""

import threading
from operator import add as _operator_add

import numpy as np

BINS = 8
B, C, H, W = 32, 32, 160, 160
HW = H * W  # 25600
NCORES = 8
BPC = B // NCORES  # 4 batches per core
PX = HW // 128  # 200 pixels per partition per batch
NJ = 4

_lock = threading.Lock()
_cache: dict = {}


def _register_hat_op():
    """Register the fused hat*logit+reduce custom DVE op (idempotent)."""
    import concourse.dve_ops as dve_ops
    from concourse.dve_spec import (
        C0,
        C1,
        PageIdx,
        Spec,
        Src0,
        Src1,
        Zero,
        One,
        lower,
        maxx,
        relu,
    )
    from concourse.dve_uop import DveOpSpec

    name = "HAT_MUL_ACC_DFL"
    if name in dve_ops._SUB_OPCODE_FOR_NAME:
        for op in dve_ops.OPS:
            if op.name == name:
                return op

    _pg = PageIdx(C0, C1)  # idx = s0 + s1*page  (page = bin k)
    _d = Src0 - _pg

    def _ref(in0, in1, s0, s1, imm2):
        P, S, N = in0.shape
        idx = (s0 + s1 * np.arange(S)).reshape(1, S, 1)
        hat = np.maximum(1.0 - np.abs(in0.astype(np.float32) - idx), 0.0)
        body = (hat * in1).astype(np.float32)
        return body, body.reshape(P, -1).sum(-1, keepdims=True)

    spec = Spec(
        body=relu(One - maxx(_d, Zero - _d)) * Src1,
        accum=_operator_add,
        accum_init=Zero,
        reference=_ref,
    )
    shas = {}
    for ver in ("v3", "v4"):
        uops = lower(spec, ver=ver)
        shas[ver] = DveOpSpec(name=name, opcode=1, uops=uops, rd1_en=True).sha(ver)
    op = dve_ops.DveOp(name, spec, subdim=True, uops_sha=shas)
    row = dve_ops._CUSTOM_DVE_ROW_BASE + len(dve_ops.OPS)
    assert row < 0x20, "custom DVE opcode rows exhausted"
    dve_ops.OPS.append(op)
    dve_ops.CUSTOM_DVE_SPECS[name] = op.spec
    dve_ops._SUB_OPCODE_FOR_NAME[name] = row
    return op


def _patch_act_tables():
    """Force Exp and Ln to resolve to the one table set containing both.

    The act-table-load pass assigns each activation the first set containing
    its function; Exp->exp_and_others and Ln->natural_log would otherwise
    alternate table loads (~1.3us each) on every exp->ln transition. Removing
    the two functions from every other set (list order and ids preserved)
    makes natural_log_exp_and_others serve both: one load for the kernel.
    """
    import concourse.bacc as bacc
    import concourse.hw_specs as hw_specs
    import concourse.mybir as mybir

    if getattr(_patch_act_tables, "_done", False):
        return
    orig = hw_specs.get_activation_tables
    Exp = mybir.ActivationFunctionType.Exp
    Ln = mybir.ActivationFunctionType.Ln

    def patched(module_arch):
        t = orig(module_arch)
        both = t.get("natural_log_exp_and_others")
        if both is not None and Exp in both and Ln in both:
            for name, fns in t.items():
                if name != "natural_log_exp_and_others":
                    fns.discard(Exp)
                    fns.discard(Ln)
        return t

    hw_specs.get_activation_tables = patched
    bacc.get_activation_tables = patched
    _patch_act_tables._done = True


def _build_nc():
    import concourse.bacc as bacc
    import concourse.mybir as mybir
    from concourse.tile import TileContext
    from concourse.dve_ops import TENSOR_TENSOR_REDUCE as ttr_op

    _patch_act_tables()
    hat_op = _register_hat_op()
    f32 = mybir.dt.float32
    bf16 = mybir.dt.bfloat16
    u8 = mybir.dt.uint8

    nc = bacc.Bacc("TRN2", target_bir_lowering=False, debug=False)
    x = nc.dram_tensor("x", [BPC, C, HW], f32, kind="ExternalInput")
    tg = nc.dram_tensor("tg", [BPC, HW, NJ], f32, kind="ExternalInput")
    mk = nc.dram_tensor("mk", [BPC, HW], u8, kind="ExternalInput")
    # acc columns: [0:16] interp (b*4+j), [16:32] lse (b*4+j), [32:36] npos100
    acc_out = nc.dram_tensor("acc", [128, 36], f32, kind="ExternalOutput")

    # DRAM views (per batch): partition p = pixel-block of 200 px
    x_v = x.rearrange("b c (blk px) -> b blk c px", px=PX)  # [4,128,32,200]
    tg_v = tg.rearrange("b (blk pj) j -> b blk (pj j)", blk=128)  # [4,128,800]
    mk_v = mk.rearrange("b (blk px) -> b blk px", px=PX)  # [4,128,200]

    Exp = mybir.ActivationFunctionType.Exp
    Ln = mybir.ActivationFunctionType.Ln
    Alu = mybir.AluOpType

    with TileContext(nc) as tc:
        with (
            tc.tile_pool(name="pL", bufs=5) as pL,
            tc.tile_pool(name="pE", bufs=4) as pE,
            tc.tile_pool(name="pS", bufs=8) as pS,
            tc.tile_pool(name="pHat", bufs=4) as pHat,
            tc.tile_pool(name="pT", bufs=3) as pT,
            tc.tile_pool(name="pOnce", bufs=1) as pOnce,
        ):
            accs = pOnce.tile([128, 36], f32)

            for b in range(BPC):
                # --- per-batch setup: targets, mask ---
                # NOTE: several TPB structs (TensorScalarPtr/STT) encode only
                # ONE sync-wait command, so each op below is arranged to need
                # at most one cross-engine semaphore wait.
                t_raw = pT.tile([128, PX * NJ], f32, tag="t_raw")  # (px, j)
                m_raw = pT.tile([128, PX], u8, tag="m_raw")
                mf100 = pT.tile([128, PX], f32, tag="mf100")
                tclp = pT.tile([128, NJ, PX], f32, tag="tclp")  # clipped t, j-major
                t2 = pT.tile([128, NJ, PX], f32, tag="t2")  # t'' j-major

                nc.sync.dma_start(t_raw[:, :], tg_v[b])
                nc.sync.dma_start(m_raw[:, :], mk_v[b])

                # prep engine: DVE for batch 0 (shortest pipeline ramp),
                # GpSimd afterwards (keeps DVE free for the custom ops)
                prep = nc.vector if b == 0 else nc.gpsimd
                # mf100 = 100 * mask (waits only on the mask DMA)
                prep.tensor_scalar(
                    out=mf100[:, :],
                    in0=m_raw[:, :],
                    scalar1=100.0,
                    scalar2=None,
                    op0=Alu.mult,
                )
                # tc = min(t, 6.9999), j-major (waits only on t DMA)
                t_raw_v = t_raw[:, :].rearrange("p (px j) -> p j px", j=NJ)
                prep.tensor_scalar(
                    out=tclp[:, :, :],
                    in0=t_raw_v,
                    scalar1=float(BINS - 1) - 1e-4,
                    scalar2=None,
                    op0=Alu.min,
                )
                # t'' = tc + 100*mask (same-engine deps only)
                prep.tensor_tensor(
                    out=t2[:, :, :],
                    in0=tclp[:, :, :],
                    in1=mf100[:, :].unsqueeze(1).broadcast_to((128, NJ, PX)),
                    op=Alu.add,
                )
                # npos accum (DVE; same-engine dep on mf100 only)
                np_scr = pT.tile([128, PX], f32, tag="np_scr")
                nc.vector.tensor_scalar(
                    out=np_scr[:, :],
                    in0=mf100[:, :],
                    scalar1=0.01,
                    scalar2=0.0,
                    op0=Alu.mult,
                    op1=Alu.add,  # reduce op for accum_out
                    accum_out=accs[:, 32 + b : 33 + b],
                )

                for j in range(NJ):
                    u = b * NJ + j
                    L = pL.tile([128, BINS, PX], f32, tag="L")
                    nc.sync.dma_start(L[:, :, :], x_v[b, :, 8 * j : 8 * j + 8, :])

                    # interp: acc[:, u] = sum_k relu(1-|t-k|) * L_k
                    hat_scr = pHat.tile([128, BINS, PX], bf16, tag="hat")
                    nc.vector._custom_dve(
                        hat_op,
                        out=hat_scr[:, :, :],
                        in0=t2[:, j, :].unsqueeze(1).broadcast_to((128, BINS, PX)),
                        in1=L[:, :, :],
                        s0=100.0,
                        s1=1.0,
                        accum_out=accs[:, u : u + 1],
                    )

                    # lse: exp -> pairwise tree -> ln -> masked accumulate
                    E = pE.tile([128, BINS, PX], bf16, tag="E")
                    nc.scalar.activation(E[:, :, :], L[:, :, :], Exp)
                    s16 = pS.tile([128, 4, PX], bf16, tag="s16")
                    s16_eng = nc.gpsimd if (u % 2 == 0) else nc.vector
                    s16_eng.tensor_tensor(
                        out=s16[:, :, :],
                        in0=E[:, 0::2, :],
                        in1=E[:, 1::2, :],
                        op=Alu.add,
                    )
                    s8 = pS.tile([128, 2, PX], bf16, tag="s8")
                    s8_eng = nc.gpsimd if (u % 4 == 1) else nc.vector
                    s8_eng.tensor_tensor(
                        out=s8[:, :, :],
                        in0=s16[:, 0::2, :],
                        in1=s16[:, 1::2, :],
                        op=Alu.add,
                    )
                    s4 = pS.tile([128, PX], f32, tag="s4")
                    nc.vector.tensor_tensor(
                        out=s4[:, :],
                        in0=s8[:, 0, :],
                        in1=s8[:, 1, :],
                        op=Alu.add,
                    )
                    lse = pS.tile([128, PX], f32, tag="lse")
                    nc.scalar.activation(lse[:, :], s4[:, :], Ln)
                    # acc[:, 16+u] = sum(lse * mf100 * 0.01) = sum(lse * mask)
                    lse_scr = pS.tile([128, PX], f32, tag="lse_scr")
                    nc.vector._custom_dve(
                        ttr_op,
                        out=lse_scr[:, :],
                        in0=lse[:, :],
                        in1=mf100[:, :],
                        s0=0.0,
                        s1=0.01,
                        accum_out=accs[:, 16 + u : 17 + u],
                    )

            nc.sync.dma_start(acc_out[:, :], accs[:, :])

    nc.finalize()
    return nc


def _get_nc():
    with _lock:
        if "nc" not in _cache:
            _cache["nc"] = _build_nc()
        return _cache["nc"]


def kernel(reg_logits: np.ndarray, targets: np.ndarray, pos_mask: np.ndarray) -> np.ndarray:
    from concourse.bass_utils import run_bass_kernel_spmd

    nc = _get_nc()

    reg_logits = np.ascontiguousarray(reg_logits, dtype=np.float32).reshape(B, C, HW)
    targets = np.ascontiguousarray(targets, dtype=np.float32)
    mask_u8 = np.ascontiguousarray(pos_mask).astype(np.uint8)

    in_maps = []
    for c in range(NCORES):
        b0 = c * BPC
        in_maps.append(
            {
                "x": reg_logits[b0 : b0 + BPC],
                "tg": targets[b0 : b0 + BPC],
                "mk": mask_u8[b0 : b0 + BPC],
            }
        )

    res = run_bass_kernel_spmd(nc, in_maps, core_ids=list(range(NCORES)))

    tot_interp = 0.0
    tot_lse = 0.0
    npos100 = 0.0
    for r in res.results:
        a = r["acc"].astype(np.float64)
        tot_interp += a[:, :16].sum()
        tot_lse += a[:, 16:32].sum()
        npos100 += a[:, 32:36].sum()

    npos = npos100  # npos accum already scaled to counts
    total = tot_lse - tot_interp
    loss = total / (max(npos, 1.0) * 4.0) if npos > 0 else 0.0
    return np.float32(loss)


if __name__ == "__main__":
    rng = np.random.default_rng(0)
    rl = rng.standard_normal((B, C, H, W), dtype=np.float32)
    tg = (rng.random((B, HW, NJ), dtype=np.float32) * (BINS - 1)).astype(np.float32)
    pm = rng.integers(0, 2, size=(B, HW)).astype(bool)
    print(kernel(reg_logits=rl, targets=tg, pos_mask=pm))



# revision 39
# speedup vs baseline: 1.0328x; 1.0328x over previous
"""DFL loss (nn_DFLLoss) Trainium2 Bass kernel — 8-core data parallel.

reference computes, per (batch, pixel, coord j in 0..3):
    rl[b, hw, j, k] = reg_logits[b, j*8+k, hw]          (k in 0..7 bins)
    t = clip(targets, 0, 6.9999); l = floor(t); u = l+1
    per = w_l * (lse - rl[l]) + w_u * (lse - rl[u]),  lse = logsumexp_k rl
    loss = sum(per * pos_mask) / (max(sum(pos_mask), 1) * 4)

Key identity used here (removes the gather):
    w_l*rl[l] + w_u*rl[u] = sum_k relu(1 - |t - k|) * rl[k]
so masked_total = sum(mask*lse) - sum_k relu(1-|t-k|)*rl[k]*mask. The
hat-product+reduce runs as ONE fused custom DVE op per (batch, coord)
with the bin index k supplied by PageIdx over the 8 channel pages.
The mask is folded into t'' = clip(t) + 100*mask and the op evaluates
relu(1 - |t'' - (100 + k)|): positive pixels give |t - k|, masked-out
pixels give |t - 100 - k| >= 93 so every hat weight is exactly 0.

Engine budget (per core, 4 batches x 4 coords = 16 units):
  DMA    41.6us  <- the memory-roofline pole (14.85 MB f32 at 360 B/ns)
  DVE    ~39us   16 custom hat ops (1x only; custom DVE ops have no fast
                 modes) + first two bin-sum tree levels as 2x bf16
                 tensor_tensor + batch-0 prep during the DMA ramp
  Act    ~31us   exp + ln (no fast modes exist on Act)
  Pool   ~20us   prep batches 1-3, final tree level, masked-lse + npos
                 accumulations
Everything except the DMA stream and its ramp/tail overlaps under it.
"""

import threading
from operator import add as _operator_add

import numpy as np

BINS = 8
B, C, H, W = 32, 32, 160, 160
HW = H * W  # 25600
NCORES = 8
BPC = B // NCORES  # 4 batches per core
PX = HW // 128  # 200 pixels per partition per batch
NJ = 4

_lock = threading.Lock()
_cache: dict = {}


def _register_hat_op():
    """Register the fused hat*logit+reduce custom DVE op (idempotent)."""
    import concourse.dve_ops as dve_ops
    from concourse.dve_spec import (
        C0,
        C1,
        PageIdx,
        Spec,
        Src0,
        Src1,
        Zero,
        One,
        lower,
        maxx,
        relu,
    )
    from concourse.dve_uop import DveOpSpec

    name = "HAT_MUL_ACC_DFL"
    if name in dve_ops._SUB_OPCODE_FOR_NAME:
        for op in dve_ops.OPS:
            if op.name == name:
                return op

    _pg = PageIdx(C0, C1)  # idx = s0 + s1*page  (page = bin k)
    _d = Src0 - _pg

    def _ref(in0, in1, s0, s1, imm2):
        P, S, N = in0.shape
        idx = (s0 + s1 * np.arange(S)).reshape(1, S, 1)
        hat = np.maximum(1.0 - np.abs(in0.astype(np.float32) - idx), 0.0)
        body = (hat * in1).astype(np.float32)
        return body, body.reshape(P, -1).sum(-1, keepdims=True)

    spec = Spec(
        body=relu(One - maxx(_d, Zero - _d)) * Src1,
        accum=_operator_add,
        accum_init=Zero,
        reference=_ref,
    )
    shas = {}
    for ver in ("v3", "v4"):
        uops = lower(spec, ver=ver)
        shas[ver] = DveOpSpec(name=name, opcode=1, uops=uops, rd1_en=True).sha(ver)
    op = dve_ops.DveOp(name, spec, subdim=True, uops_sha=shas)
    row = dve_ops._CUSTOM_DVE_ROW_BASE + len(dve_ops.OPS)
    assert row < 0x20, "custom DVE opcode rows exhausted"
    dve_ops.OPS.append(op)
    dve_ops.CUSTOM_DVE_SPECS[name] = op.spec
    dve_ops._SUB_OPCODE_FOR_NAME[name] = row
    return op


def _patch_act_tables():
    """Force Exp and Ln to resolve to the one table set containing both.

    The act-table-load pass assigns each activation the first set containing
    its function; Exp->exp_and_others and Ln->natural_log would otherwise
    alternate table loads (~1.3us each) on every exp->ln transition. Removing
    the two functions from every other set (list order and ids preserved)
    makes natural_log_exp_and_others serve both: one load for the kernel.
    """
    import concourse.bacc as bacc
    import concourse.hw_specs as hw_specs
    import concourse.mybir as mybir

    if getattr(_patch_act_tables, "_done", False):
        return
    orig = hw_specs.get_activation_tables
    Exp = mybir.ActivationFunctionType.Exp
    Ln = mybir.ActivationFunctionType.Ln

    def patched(module_arch):
        t = orig(module_arch)
        both = t.get("natural_log_exp_and_others")
        if both is not None and Exp in both and Ln in both:
            for name, fns in t.items():
                if name != "natural_log_exp_and_others":
                    fns.discard(Exp)
                    fns.discard(Ln)
        return t

    hw_specs.get_activation_tables = patched
    bacc.get_activation_tables = patched
    _patch_act_tables._done = True


def _build_nc():
    import concourse.bacc as bacc
    import concourse.mybir as mybir
    from concourse.tile import TileContext

    _patch_act_tables()
    hat_op = _register_hat_op()
    f32 = mybir.dt.float32
    bf16 = mybir.dt.bfloat16
    u8 = mybir.dt.uint8

    nc = bacc.Bacc("TRN2", target_bir_lowering=False, debug=False)
    x = nc.dram_tensor("x", [BPC, C, HW], f32, kind="ExternalInput")
    tg = nc.dram_tensor("tg", [BPC, HW, NJ], f32, kind="ExternalInput")
    mk = nc.dram_tensor("mk", [BPC, HW], u8, kind="ExternalInput")
    # acc columns: [0:16] interp (unit u = b*4+j), [16:19] lse for batch
    # groups b0/b1/b2 (masked-sum via STT), [19] lse group (units 12,13),
    # [20] lse unit 14, [21] lse unit 15 (the latter three via the mask-blend
    # s4' = max(min(s4, m?3e38:1), 1-m) so ln(s4') accumulates pre-masked),
    # [32:36] npos per batch, [36:40] second-half interp for bin-split units
    acc_out = nc.dram_tensor("acc", [128, 40], f32, kind="ExternalOutput")

    # DRAM views (per batch): partition p = pixel-block of 200 px
    x_v = x.rearrange("b c (blk px) -> b blk c px", px=PX)  # [4,128,32,200]
    tg_v = tg.rearrange("b (blk pj) j -> b blk (pj j)", blk=128)  # [4,128,800]
    mk_v = mk.rearrange("b (blk px) -> b blk px", px=PX)  # [4,128,200]

    Exp = mybir.ActivationFunctionType.Exp
    Ln = mybir.ActivationFunctionType.Ln
    Alu = mybir.AluOpType

    with TileContext(nc) as tc:
        with (
            tc.tile_pool(name="pL", bufs=5) as pL,
            tc.tile_pool(name="pE", bufs=4) as pE,
            tc.tile_pool(name="pS", bufs=8) as pS,
            tc.tile_pool(name="pHat", bufs=2) as pHat,
            tc.tile_pool(name="pT", bufs=2) as pT,
            tc.tile_pool(name="pOnce", bufs=1) as pOnce,
        ):
            accs = pOnce.tile([128, 40], f32)
            nc.gpsimd.memset(accs[:, :], 0.0)  # unused cols must read as 0
            mf100s: dict = {}
            t2s: dict = {}
            tails: dict = {}
            s4gs: dict = {}

            def eng_add(eng, out, in0, in1):
                """Plain adds; scalar_tensor_tensor is NOT a legal Pool
                opcode in walrus codegen, so both engines use tensor_tensor
                (2x bf16 on DVE, Add-efficiency 0.42 on Pool)."""
                eng.tensor_tensor(out=out, in0=in0, in1=in1, op=Alu.add)

            def emit_batch_prep(b, tclp_eng=None):
                """mk/tg DMAs + mask/target prep. Emitted one unit ahead of
                the batch's first L so t'' is ready for its hats.
                Mask trick: t'' = clip(t) - 100*(1-m) so positive pixels
                keep t exactly (bf16-safe near 0..7) and masked pixels sit
                ~100 below every bin index (hat weight exactly 0; s0=0)."""
                t_raw = pT.tile([128, PX * NJ], f32, tag="t_raw")  # (px, j)
                m_raw = pT.tile([128, PX], u8, tag="m_raw")
                m1 = pT.tile([128, PX], bf16, tag="m1")
                mc100 = pT.tile([128, PX], bf16, tag="mc100")
                tclp = pT.tile([128, NJ, PX], bf16, tag="tclp")
                t2 = pT.tile([128, NJ, PX], bf16, tag="t2")
                mf100s[b] = m1
                t2s[b] = t2
                nc.sync.dma_start(m_raw[:, :], mk_v[b])
                nc.sync.dma_start(t_raw[:, :], tg_v[b])
                # m1 = mask as 1.0/0.0 (exact in bf16)
                nc.gpsimd.tensor_scalar(
                    out=m1[:, :],
                    in0=m_raw[:, :],
                    scalar1=1.0,
                    scalar2=None,
                    op0=Alu.mult,
                )
                # mc100 = -100*(1-m) = 100*m - 100 (0 or -100, both exact)
                nc.gpsimd.tensor_scalar(
                    out=mc100[:, :],
                    in0=m_raw[:, :],
                    scalar1=100.0,
                    scalar2=-100.0,
                    op0=Alu.mult,
                    op1=Alu.add,
                )
                # tclp = min(t, 6.9999), j-major, bf16
                t_raw_v = t_raw[:, :].rearrange("p (px j) -> p j px", j=NJ)
                (tclp_eng or nc.gpsimd).tensor_scalar(
                    out=tclp[:, :, :],
                    in0=t_raw_v,
                    scalar1=float(BINS - 1) - 1e-4,
                    scalar2=None,
                    op0=Alu.min,
                )
                # t'' = tclp + mc100 (DVE 2x bf16; adding 0 is exact)
                nc.vector.tensor_tensor(
                    out=t2[:, :, :],
                    in0=tclp[:, :, :],
                    in1=mc100[:, :].unsqueeze(1).broadcast_to((128, NJ, PX)),
                    op=Alu.add,
                )
                # npos accum: sum(m1) (DVE, 4x bf16)
                np_scr = pT.tile([128, PX], bf16, tag="np_scr")
                nc.vector.tensor_scalar(
                    out=np_scr[:, :],
                    in0=m1[:, :],
                    scalar1=1.0,
                    scalar2=0.0,
                    op0=Alu.mult,
                    op1=Alu.add,  # reduce op for accum_out
                    accum_out=accs[:, 32 + b : 33 + b],
                )
                # group s4 buffer for this batch's lse values
                nj = NJ if b < 3 else NJ - 2  # batch 3: 14/15 handled apart
                s4g = pS.tile([128, nj, PX], bf16, tag="s4g")
                s4gs[b] = s4g

            def emit_unit_head(u):
                """DMA + hat + exp for a full unit."""
                b, j = divmod(u, NJ)
                L = pL.tile([128, BINS, PX], f32, tag="L")
                nc.sync.dma_start(L[:, :, :], x_v[b, :, 8 * j : 8 * j + 8, :])
                # interp: acc[:, u] = sum_k relu(1-|t-k|) * L_k  (DVE, 1x)
                hat_scr = pHat.tile([128, BINS, PX], bf16, tag="hat")
                nc.vector._custom_dve(
                    hat_op,
                    out=hat_scr[:, :, :],
                    in0=t2s[b][:, j, :].unsqueeze(1).broadcast_to((128, BINS, PX)),
                    in1=L[:, :, :],
                    s0=0.0,
                    s1=1.0,
                    accum_out=accs[:, u : u + 1],
                )
                E = pE.tile([128, BINS, PX], bf16, tag="E")
                nc.scalar.activation(E[:, :, :], L[:, :, :], Exp)
                tails[u] = E

            def emit_unit_tree(u, eng, s4_eng=None):
                """Lagged tree levels; s4 lands in the batch group buffer.
                The final (cheap) level defaults to Pool to unload DVE."""
                E = tails[u]
                b, j = divmod(u, NJ)
                s16 = pS.tile([128, 4, PX], bf16, tag="s16")
                eng_add(eng, s16[:, :, :], E[:, 0::2, :], E[:, 1::2, :])
                s8 = pS.tile([128, 2, PX], bf16, tag="s8")
                eng_add(eng, s8[:, :, :], s16[:, 0::2, :], s16[:, 1::2, :])
                s4e = s4_eng or nc.gpsimd
                eng_add(s4e, s4gs[b][:, j, :], s8[:, 0, :], s8[:, 1, :])

            lses: dict = {}

            def emit_group_lse(b, nj=None):
                """One ln for a batch group (slot into the boundary hole)."""
                s4g = s4gs[b]
                nj = nj or s4g.shape[1]
                lse = pS.tile([128, nj, PX], bf16, tag="lseg")
                nc.scalar.activation(lse[:, :, :], s4g[:, 0:nj, :], Ln)
                lses[b] = lse

            def emit_group_macc(b, eng=None):
                """Masked lse accumulate (mult then scalar accum);
                emitted lagged so it never head-of-line blocks a hat."""
                eng = eng or nc.vector
                lse = lses[b]
                nj = lse.shape[1]
                lsem = pS.tile([128, nj, PX], bf16, tag="lsem")
                eng.tensor_tensor(
                    out=lsem[:, :, :],
                    in0=lse[:, :, :],
                    in1=mf100s[b][:, :].unsqueeze(1).broadcast_to((128, nj, PX)),
                    op=Alu.mult,
                )
                lse_scr = pS.tile([128, nj, PX], bf16, tag="lse_scr")
                eng.tensor_scalar(
                    out=lse_scr[:, :, :],
                    in0=lsem[:, :, :],
                    scalar1=1.0,
                    scalar2=0.0,
                    op0=Alu.mult,
                    op1=Alu.add,
                    accum_out=accs[:, 16 + b : 17 + b],
                )

            def emit_half(u, h, acc_col, tree_eng=None):
                """One bin-half (4 bins) of a split unit: DMA + hat + exp +
                its half-tree down to a [128, PX] partial sum (returned)."""
                b, j = divmod(u, NJ)
                c0 = 8 * j + 4 * h
                tree_eng = tree_eng or nc.vector
                Lh = pL.tile([128, 4, PX], f32, tag="Lh")
                nc.sync.dma_start(Lh[:, :, :], x_v[b, :, c0 : c0 + 4, :])
                hat_scr = pHat.tile([128, 4, PX], bf16, tag="hath")
                nc.vector._custom_dve(
                    hat_op,
                    out=hat_scr[:, :, :],
                    in0=t2s[b][:, j, :].unsqueeze(1).broadcast_to((128, 4, PX)),
                    in1=Lh[:, :, :],
                    s0=4.0 * h,
                    s1=1.0,
                    accum_out=accs[:, acc_col : acc_col + 1],
                )
                Eh = pE.tile([128, 4, PX], bf16, tag="Eh")
                nc.scalar.activation(Eh[:, :, :], Lh[:, :, :], Exp)
                s16h = pS.tile([128, 2, PX], bf16, tag="s16h")
                eng_add(tree_eng, s16h[:, :, :], Eh[:, 0::2, :], Eh[:, 1::2, :])
                s8h = pS.tile([128, PX], bf16, tag="s8h")
                eng_add(tree_eng, s8h[:, :], s16h[:, 0, :], s16h[:, 1, :])
                return s8h

            def emit_clamps(src, dst_slice, nj, clamp_eng):
                """Mask-blend s4' = max(min(s4, hi3), lo3) into dst_slice:
                positives keep s4, masked pixels become exactly 1 (ln -> 0)."""
                mn = pS.tile([128, nj, PX], bf16, tag="mn")
                hib = hi3[:, :].unsqueeze(1).broadcast_to((128, nj, PX))
                lob = lo3[:, :].unsqueeze(1).broadcast_to((128, nj, PX))
                clamp_eng.tensor_tensor(out=mn[:, :, :], in0=src, in1=hib, op=Alu.min)
                clamp_eng.tensor_tensor(out=dst_slice, in0=mn[:, :, :], in1=lob, op=Alu.max)

            # --- emission: software-pipelined over units -------------------
            # Unit 0 is bin-halved purely to start hat/exp ~1.2us earlier
            # (DVE computes t2_0 in the otherwise idle ramp).
            emit_batch_prep(0, tclp_eng=nc.vector)
            h0 = emit_half(0, 0, 0)
            h1 = emit_half(0, 1, 36)
            nc.vector.tensor_tensor(
                out=s4gs[0][:, 0, :], in0=h0[:, :], in1=h1[:, :], op=Alu.add
            )

            # Trees of mid-batch units go to Pool; the j==3 unit's tree is
            # emitted un-lagged so the batch group's ln+macc slot into the
            # Act hole at the batch boundary (mk/tg DMAs delay the next L).
            POOL_TREES = {2, 4, 6, 8, 10, 12}
            for u in range(1, 14):
                b, j = divmod(u, NJ)
                if j == 2 and b < 3:
                    emit_batch_prep(b + 1)
                emit_unit_head(u)
                prev = u - 1
                if prev >= 1 and prev % NJ != 3:
                    eng = nc.gpsimd if prev in POOL_TREES else nc.vector
                    emit_unit_tree(prev, eng)
                if j == 3:
                    # un-lagged tree; s4 on Pool like every other group-tile
                    # writer (cross-engine slice writes serialize falsely)
                    emit_unit_tree(u, nc.vector)
                    emit_group_lse(b)  # whole batch b, in the boundary hole
                elif j == 1 and b >= 1:
                    emit_group_macc(b - 1)

            # batch-3 mask-blend clamp tensors (hi = m ? 3e38 : 1, lo = 1-m)
            hi3 = pOnce.tile([128, PX], bf16)
            nc.gpsimd.tensor_scalar(
                out=hi3[:, :],
                in0=mf100s[3][:, :],
                scalar1=3e38,
                scalar2=1.0,
                op0=Alu.mult,
                op1=Alu.add,
            )
            lo3 = pOnce.tile([128, PX], bf16)
            nc.gpsimd.tensor_scalar(
                out=lo3[:, :],
                in0=mf100s[3][:, :],
                scalar1=-1.0,
                scalar2=1.0,
                op0=Alu.mult,
                op1=Alu.add,
            )
            # clamped s4 buffer for units 14/15 -> one ln-accum (col 20)
            s4c = pS.tile([128, 2, PX], bf16, tag="s4c")

            # units 14 and 15 bin-halved; tail trees split across DVE/Pool
            ha = emit_half(14, 0, 14)           # DVE half-tree
            emit_unit_tree(13, nc.vector)       # lagged tree(13); s4 -> Pool
            hb = emit_half(14, 1, 38, tree_eng=nc.gpsimd)
            ga = emit_half(15, 0, 15)           # DVE half-tree
            # s4_14 join + clamps on Pool into s4c[:,0,:]
            s4_14 = pS.tile([128, 1, PX], bf16, tag="s4")
            eng_add(nc.gpsimd, s4_14[:, 0, :], ha[:, :], hb[:, :])
            emit_clamps(s4_14[:, :, :], s4c[:, 0:1, :], 1, clamp_eng=nc.vector)
            gb = emit_half(15, 1, 39)           # DVE half-tree
            # unit 15 join + clamps on DVE into s4c[:,1,:]
            s4_15 = pS.tile([128, 1, PX], bf16, tag="s4")
            nc.vector.tensor_tensor(
                out=s4_15[:, 0, :], in0=ga[:, :], in1=gb[:, :], op=Alu.add
            )
            emit_clamps(s4_15[:, :, :], s4c[:, 1:2, :], 1, clamp_eng=nc.vector)
            emit_group_macc(2)  # batch 2 masked accumulate (lagged)
            # group (12,13) masked-sum; emitted AFTER exp15b so it cannot
            # head-of-line block the last exp in Act's in-order queue
            emit_group_lse(3)  # units 12-13 -> Ln
            emit_group_macc(3)  # -> col 19
            # single ln-accum over clamped units 14+15 -> col 20
            lse_c = pS.tile([128, 2, PX], bf16, tag="lseg")
            nc.scalar.activation(
                lse_c[:, :, :], s4c[:, :, :], Ln, accum_out=accs[:, 20:21]
            )

            nc.sync.dma_start(acc_out[:, :], accs[:, :])

    nc.finalize()
    return nc


def _get_nc():
    with _lock:
        if "nc" not in _cache:
            _cache["nc"] = _build_nc()
        return _cache["nc"]


def kernel(reg_logits: np.ndarray, targets: np.ndarray, pos_mask: np.ndarray) -> np.ndarray:
    from concourse.bass_utils import run_bass_kernel_spmd

    nc = _get_nc()

    reg_logits = np.ascontiguousarray(reg_logits, dtype=np.float32).reshape(B, C, HW)
    targets = np.ascontiguousarray(targets, dtype=np.float32)
    mask_u8 = np.ascontiguousarray(pos_mask).astype(np.uint8)

    in_maps = []
    for c in range(NCORES):
        b0 = c * BPC
        in_maps.append(
            {
                "x": reg_logits[b0 : b0 + BPC],
                "tg": targets[b0 : b0 + BPC],
                "mk": mask_u8[b0 : b0 + BPC],
            }
        )

    res = run_bass_kernel_spmd(nc, in_maps, core_ids=list(range(NCORES)))

    tot_interp = 0.0
    tot_lse = 0.0
    npos = 0.0
    for r in res.results:
        a = r["acc"].astype(np.float64)
        tot_interp += a[:, :16].sum() + a[:, 36:40].sum()
        tot_lse += a[:, 16:21].sum()
        npos += a[:, 32:36].sum()

    total = tot_lse - tot_interp
    loss = total / (max(npos, 1.0) * 4.0) if npos > 0 else 0.0
    return np.float32(loss)


if __name__ == "__main__":
    rng = np.random.default_rng(0)
    rl = rng.standard_normal((B, C, H, W), dtype=np.float32)
    tg = (rng.random((B, HW, NJ), dtype=np.float32) * (BINS - 1)).astype(np.float32)
    pm = rng.integers(0, 2, size=(B, HW)).astype(bool)
    print(kernel(reg_logits=rl, targets=tg, pos_mask=pm))


# revision 48
# speedup vs baseline: 1.0449x; 1.0117x over previous
"""DFL loss (nn_DFLLoss) Trainium2 Bass kernel — 8-core data parallel.

reference computes, per (batch, pixel, coord j in 0..3):
    rl[b, hw, j, k] = reg_logits[b, j*8+k, hw]          (k in 0..7 bins)
    t = clip(targets, 0, 6.9999); l = floor(t); u = l+1
    per = w_l * (lse - rl[l]) + w_u * (lse - rl[u]),  lse = logsumexp_k rl
    loss = sum(per * pos_mask) / (max(sum(pos_mask), 1) * 4)

Key identity used here (removes the gather):
    w_l*rl[l] + w_u*rl[u] = sum_k relu(1 - |t - k|) * rl[k]
so masked_total = sum(mask*lse) - sum_k relu(1-|t-k|)*rl[k]*mask. The
hat-product+reduce runs as ONE fused custom DVE op per (batch, coord)
with the bin index k supplied by PageIdx over the 8 channel pages.
The mask is folded into t'' = clip(t) + 100*mask and the op evaluates
relu(1 - |t'' - (100 + k)|): positive pixels give |t - k|, masked-out
pixels give |t - 100 - k| >= 93 so every hat weight is exactly 0.

Engine budget (per core, 4 batches x 4 coords = 16 units):
  DMA    41.6us  <- the memory-roofline pole (14.85 MB f32 at 360 B/ns)
  DVE    ~39us   16 custom hat ops (1x only; custom DVE ops have no fast
                 modes) + first two bin-sum tree levels as 2x bf16
                 tensor_tensor + batch-0 prep during the DMA ramp
  Act    ~31us   exp + ln (no fast modes exist on Act)
  Pool   ~20us   prep batches 1-3, final tree level, masked-lse + npos
                 accumulations
Everything except the DMA stream and its ramp/tail overlaps under it.
"""

import threading
from operator import add as _operator_add

import numpy as np

BINS = 8
B, C, H, W = 32, 32, 160, 160
HW = H * W  # 25600
NCORES = 8
BPC = B // NCORES  # 4 batches per core
PX = HW // 128  # 200 pixels per partition per batch
NJ = 4

_lock = threading.Lock()
_cache: dict = {}


def _register_hat_op():
    """Register the fused hat*logit+reduce custom DVE op (idempotent)."""
    import concourse.dve_ops as dve_ops
    from concourse.dve_spec import (
        C0,
        C1,
        PageIdx,
        Spec,
        Src0,
        Src1,
        Zero,
        One,
        lower,
        maxx,
        relu,
    )
    from concourse.dve_uop import DveOpSpec

    name = "HAT_MUL_ACC_DFL"
    if name in dve_ops._SUB_OPCODE_FOR_NAME:
        for op in dve_ops.OPS:
            if op.name == name:
                return op

    _pg = PageIdx(C0, C1)  # idx = s0 + s1*page  (page = bin k)
    _d = Src0 - _pg

    def _ref(in0, in1, s0, s1, imm2):
        P, S, N = in0.shape
        idx = (s0 + s1 * np.arange(S)).reshape(1, S, 1)
        hat = np.maximum(1.0 - np.abs(in0.astype(np.float32) - idx), 0.0)
        body = (hat * in1).astype(np.float32)
        return body, body.reshape(P, -1).sum(-1, keepdims=True)

    spec = Spec(
        body=relu(One - maxx(_d, Zero - _d)) * Src1,
        accum=_operator_add,
        accum_init=Zero,
        reference=_ref,
    )
    shas = {}
    for ver in ("v3", "v4"):
        uops = lower(spec, ver=ver)
        shas[ver] = DveOpSpec(name=name, opcode=1, uops=uops, rd1_en=True).sha(ver)
    op = dve_ops.DveOp(name, spec, subdim=True, uops_sha=shas)
    row = dve_ops._CUSTOM_DVE_ROW_BASE + len(dve_ops.OPS)
    assert row < 0x20, "custom DVE opcode rows exhausted"
    dve_ops.OPS.append(op)
    dve_ops.CUSTOM_DVE_SPECS[name] = op.spec
    dve_ops._SUB_OPCODE_FOR_NAME[name] = row
    return op


def _patch_act_tables():
    """Force Exp and Ln to resolve to the one table set containing both.

    The act-table-load pass assigns each activation the first set containing
    its function; Exp->exp_and_others and Ln->natural_log would otherwise
    alternate table loads (~1.3us each) on every exp->ln transition. Removing
    the two functions from every other set (list order and ids preserved)
    makes natural_log_exp_and_others serve both: one load for the kernel.
    """
    import concourse.bacc as bacc
    import concourse.hw_specs as hw_specs
    import concourse.mybir as mybir

    if getattr(_patch_act_tables, "_done", False):
        return
    orig = hw_specs.get_activation_tables
    Exp = mybir.ActivationFunctionType.Exp
    Ln = mybir.ActivationFunctionType.Ln

    def patched(module_arch):
        t = orig(module_arch)
        both = t.get("natural_log_exp_and_others")
        if both is not None and Exp in both and Ln in both:
            for name, fns in t.items():
                if name != "natural_log_exp_and_others":
                    fns.discard(Exp)
                    fns.discard(Ln)
        return t

    hw_specs.get_activation_tables = patched
    bacc.get_activation_tables = patched
    _patch_act_tables._done = True


def _build_nc():
    import concourse.bacc as bacc
    import concourse.mybir as mybir
    from concourse.tile import TileContext

    _patch_act_tables()
    hat_op = _register_hat_op()
    f32 = mybir.dt.float32
    bf16 = mybir.dt.bfloat16
    u8 = mybir.dt.uint8

    nc = bacc.Bacc("TRN2", target_bir_lowering=False, debug=False)
    x = nc.dram_tensor("x", [BPC, C, HW], f32, kind="ExternalInput")
    tg = nc.dram_tensor("tg", [BPC, HW, NJ], f32, kind="ExternalInput")
    mk = nc.dram_tensor("mk", [BPC, HW], u8, kind="ExternalInput")
    # acc columns: [0:16] interp (unit u = b*4+j), [16:19] lse for batch
    # groups b0/b1/b2 (masked-sum via STT), [19] lse group (units 12,13),
    # [20] lse unit 14, [21] lse unit 15 (the latter three via the mask-blend
    # s4' = max(min(s4, m?3e38:1), 1-m) so ln(s4') accumulates pre-masked),
    # [32:36] npos per batch, [36:40] second-half interp for bin-split units
    acc_out = nc.dram_tensor("acc", [128, 40], f32, kind="ExternalOutput")

    # DRAM views (per batch): partition p = pixel-block of 200 px
    x_v = x.rearrange("b c (blk px) -> b blk c px", px=PX)  # [4,128,32,200]
    tg_v = tg.rearrange("b (blk pj) j -> b blk (pj j)", blk=128)  # [4,128,800]
    mk_v = mk.rearrange("b (blk px) -> b blk px", px=PX)  # [4,128,200]

    Exp = mybir.ActivationFunctionType.Exp
    Ln = mybir.ActivationFunctionType.Ln
    Alu = mybir.AluOpType

    with TileContext(nc) as tc:
        with (
            tc.tile_pool(name="pL", bufs=5) as pL,
            tc.tile_pool(name="pE", bufs=4) as pE,
            tc.tile_pool(name="pS", bufs=8) as pS,
            tc.tile_pool(name="pHat", bufs=2) as pHat,
            tc.tile_pool(name="pT", bufs=2) as pT,
            tc.tile_pool(name="pOnce", bufs=1) as pOnce,
        ):
            accs = pOnce.tile([128, 40], f32)
            nc.gpsimd.memset(accs[:, :], 0.0)  # unused cols must read as 0
            mf100s: dict = {}
            t2s: dict = {}
            tails: dict = {}
            s4gs: dict = {}

            def eng_add(eng, out, in0, in1):
                """Plain adds; scalar_tensor_tensor is NOT a legal Pool
                opcode in walrus codegen, so both engines use tensor_tensor
                (2x bf16 on DVE, Add-efficiency 0.42 on Pool)."""
                eng.tensor_tensor(out=out, in0=in0, in1=in1, op=Alu.add)

            def emit_batch_prep(b, tclp_eng=None):
                """mk/tg DMAs + mask/target prep. Emitted one unit ahead of
                the batch's first L so t'' is ready for its hats.
                Mask trick: t'' = clip(t) - 100*(1-m) so positive pixels
                keep t exactly (bf16-safe near 0..7) and masked pixels sit
                ~100 below every bin index (hat weight exactly 0; s0=0)."""
                t_raw = pT.tile([128, PX * NJ], f32, tag="t_raw")  # (px, j)
                m_raw = pT.tile([128, PX], u8, tag="m_raw")
                m1 = pT.tile([128, PX], bf16, tag="m1")
                mc100 = pT.tile([128, PX], bf16, tag="mc100")
                tclp = pT.tile([128, NJ, PX], bf16, tag="tclp")
                t2 = pT.tile([128, NJ, PX], bf16, tag="t2")
                mf100s[b] = m1
                t2s[b] = t2
                nc.sync.dma_start(m_raw[:, :], mk_v[b])
                nc.sync.dma_start(t_raw[:, :], tg_v[b])
                # m1 = mask as 1.0/0.0 (exact in bf16)
                nc.gpsimd.tensor_scalar(
                    out=m1[:, :],
                    in0=m_raw[:, :],
                    scalar1=1.0,
                    scalar2=None,
                    op0=Alu.mult,
                )
                # mc100 = -100*(1-m) = 100*m - 100 (0 or -100, both exact)
                nc.gpsimd.tensor_scalar(
                    out=mc100[:, :],
                    in0=m_raw[:, :],
                    scalar1=100.0,
                    scalar2=-100.0,
                    op0=Alu.mult,
                    op1=Alu.add,
                )
                # tclp = min(t, 6.9999), j-major, bf16
                t_raw_v = t_raw[:, :].rearrange("p (px j) -> p j px", j=NJ)
                (tclp_eng or nc.gpsimd).tensor_scalar(
                    out=tclp[:, :, :],
                    in0=t_raw_v,
                    scalar1=float(BINS - 1) - 1e-4,
                    scalar2=None,
                    op0=Alu.min,
                )
                # t'' = tclp + mc100 (DVE 2x bf16; adding 0 is exact)
                nc.vector.tensor_tensor(
                    out=t2[:, :, :],
                    in0=tclp[:, :, :],
                    in1=mc100[:, :].unsqueeze(1).broadcast_to((128, NJ, PX)),
                    op=Alu.add,
                )
                # npos accum: sum(m1) (DVE, 4x bf16)
                np_scr = pT.tile([128, PX], bf16, tag="np_scr")
                nc.vector.tensor_scalar(
                    out=np_scr[:, :],
                    in0=m1[:, :],
                    scalar1=1.0,
                    scalar2=0.0,
                    op0=Alu.mult,
                    op1=Alu.add,  # reduce op for accum_out
                    accum_out=accs[:, 32 + b : 33 + b],
                )
                # group s4 buffer for this batch's lse values
                nj = NJ if b < 3 else NJ - 2  # batch 3: 14/15 handled apart
                s4g = pS.tile([128, nj, PX], bf16, tag="s4g")
                s4gs[b] = s4g

            def emit_unit_head(u):
                """DMA + hat + exp for a full unit."""
                b, j = divmod(u, NJ)
                L = pL.tile([128, BINS, PX], f32, tag="L")
                nc.sync.dma_start(L[:, :, :], x_v[b, :, 8 * j : 8 * j + 8, :])
                # interp: acc[:, u] = sum_k relu(1-|t-k|) * L_k  (DVE, 1x)
                hat_scr = pHat.tile([128, BINS, PX], bf16, tag="hat")
                nc.vector._custom_dve(
                    hat_op,
                    out=hat_scr[:, :, :],
                    in0=t2s[b][:, j, :].unsqueeze(1).broadcast_to((128, BINS, PX)),
                    in1=L[:, :, :],
                    s0=0.0,
                    s1=1.0,
                    accum_out=accs[:, u : u + 1],
                )
                E = pE.tile([128, BINS, PX], bf16, tag="E")
                nc.scalar.activation(E[:, :, :], L[:, :, :], Exp)
                tails[u] = E

            def emit_unit_tree(u, eng, s4_eng=None):
                """Lagged tree levels; s4 lands in the batch group buffer.
                The final (cheap) level defaults to Pool to unload DVE."""
                E = tails[u]
                b, j = divmod(u, NJ)
                s16 = pS.tile([128, 4, PX], bf16, tag="s16")
                eng_add(eng, s16[:, :, :], E[:, 0::2, :], E[:, 1::2, :])
                s8 = pS.tile([128, 2, PX], bf16, tag="s8")
                eng_add(eng, s8[:, :, :], s16[:, 0::2, :], s16[:, 1::2, :])
                s4e = s4_eng or nc.gpsimd
                eng_add(s4e, s4gs[b][:, j, :], s8[:, 0, :], s8[:, 1, :])

            lses: dict = {}

            def emit_group_lse(b, nj=None):
                """One ln for a batch group (slot into the boundary hole)."""
                s4g = s4gs[b]
                nj = nj or s4g.shape[1]
                lse = pS.tile([128, nj, PX], bf16, tag="lseg")
                nc.scalar.activation(lse[:, :, :], s4g[:, 0:nj, :], Ln)
                lses[b] = lse

            def emit_group_macc(b, eng=None):
                """Masked lse accumulate (mult then scalar accum);
                emitted lagged so it never head-of-line blocks a hat."""
                eng = eng or nc.vector
                lse = lses[b]
                nj = lse.shape[1]
                lsem = pS.tile([128, nj, PX], bf16, tag="lsem")
                eng.tensor_tensor(
                    out=lsem[:, :, :],
                    in0=lse[:, :, :],
                    in1=mf100s[b][:, :].unsqueeze(1).broadcast_to((128, nj, PX)),
                    op=Alu.mult,
                )
                lse_scr = pS.tile([128, nj, PX], bf16, tag="lse_scr")
                eng.tensor_scalar(
                    out=lse_scr[:, :, :],
                    in0=lsem[:, :, :],
                    scalar1=1.0,
                    scalar2=0.0,
                    op0=Alu.mult,
                    op1=Alu.add,
                    accum_out=accs[:, 16 + b : 17 + b],
                )

            def emit_half(u, h, acc_col, tree_eng=None):
                """One bin-half (4 bins) of a split unit: DMA + hat + exp +
                its half-tree down to a [128, PX] partial sum (returned)."""
                b, j = divmod(u, NJ)
                c0 = 8 * j + 4 * h
                tree_eng = tree_eng or nc.vector
                Lh = pL.tile([128, 4, PX], f32, tag="Lh")
                nc.sync.dma_start(Lh[:, :, :], x_v[b, :, c0 : c0 + 4, :])
                hat_scr = pHat.tile([128, 4, PX], bf16, tag="hath")
                nc.vector._custom_dve(
                    hat_op,
                    out=hat_scr[:, :, :],
                    in0=t2s[b][:, j, :].unsqueeze(1).broadcast_to((128, 4, PX)),
                    in1=Lh[:, :, :],
                    s0=4.0 * h,
                    s1=1.0,
                    accum_out=accs[:, acc_col : acc_col + 1],
                )
                Eh = pE.tile([128, 4, PX], bf16, tag="Eh")
                nc.scalar.activation(Eh[:, :, :], Lh[:, :, :], Exp)
                s16h = pS.tile([128, 2, PX], bf16, tag="s16h")
                eng_add(tree_eng, s16h[:, :, :], Eh[:, 0::2, :], Eh[:, 1::2, :])
                s8h = pS.tile([128, PX], bf16, tag="s8h")
                eng_add(tree_eng, s8h[:, :], s16h[:, 0, :], s16h[:, 1, :])
                return s8h

            def emit_clamps(src, dst_slice, nj, clamp_eng):
                """Mask-blend s4' = max(min(s4, hi3), lo3) into dst_slice:
                positives keep s4, masked pixels become exactly 1 (ln -> 0)."""
                mn = pS.tile([128, nj, PX], bf16, tag="mn")
                hib = hi3[:, :].unsqueeze(1).broadcast_to((128, nj, PX))
                lob = lo3[:, :].unsqueeze(1).broadcast_to((128, nj, PX))
                clamp_eng.tensor_tensor(out=mn[:, :, :], in0=src, in1=hib, op=Alu.min)
                clamp_eng.tensor_tensor(out=dst_slice, in0=mn[:, :, :], in1=lob, op=Alu.max)

            # --- emission: software-pipelined over units -------------------
            # Unit 0 is bin-halved purely to start hat/exp ~1.2us earlier
            # (DVE computes t2_0 in the otherwise idle ramp).
            emit_batch_prep(0, tclp_eng=nc.vector)
            h0 = emit_half(0, 0, 0)
            h1 = emit_half(0, 1, 36)
            nc.vector.tensor_tensor(
                out=s4gs[0][:, 0, :], in0=h0[:, :], in1=h1[:, :], op=Alu.add
            )

            # Trees of mid-batch units go to Pool; the j==3 unit's tree is
            # emitted un-lagged so the batch group's ln+macc slot into the
            # Act hole at the batch boundary (mk/tg DMAs delay the next L).
            POOL_TREES = {2, 6, 10, 12}
            for u in range(1, 14):
                b, j = divmod(u, NJ)
                if j == 2 and b < 3:
                    emit_batch_prep(b + 1)
                emit_unit_head(u)
                prev = u - 1
                if prev >= 1 and prev % NJ != 3:
                    eng = nc.gpsimd if prev in POOL_TREES else nc.vector
                    emit_unit_tree(prev, eng)
                if j == 3:
                    # un-lagged tree; s4 on Pool like every other group-tile
                    # writer (cross-engine slice writes serialize falsely)
                    emit_unit_tree(u, nc.vector)
                    emit_group_lse(b)  # whole batch b, in the boundary hole
                elif j == 1 and b >= 1:
                    emit_group_macc(b - 1)

            # batch-3 mask-blend clamp tensors (hi = m ? 3e38 : 1, lo = 1-m)
            hi3 = pOnce.tile([128, PX], bf16)
            nc.gpsimd.tensor_scalar(
                out=hi3[:, :],
                in0=mf100s[3][:, :],
                scalar1=3e38,
                scalar2=1.0,
                op0=Alu.mult,
                op1=Alu.add,
            )
            lo3 = pOnce.tile([128, PX], bf16)
            nc.gpsimd.tensor_scalar(
                out=lo3[:, :],
                in0=mf100s[3][:, :],
                scalar1=-1.0,
                scalar2=1.0,
                op0=Alu.mult,
                op1=Alu.add,
            )
            # clamped s4 buffer for units 14/15 -> one ln-accum (col 20)
            s4c = pS.tile([128, 2, PX], bf16, tag="s4c")

            # units 14 and 15 bin-halved; tail trees split across DVE/Pool
            ha = emit_half(14, 0, 14)           # DVE half-tree
            emit_unit_tree(13, nc.vector)       # lagged tree(13); s4 -> Pool
            hb = emit_half(14, 1, 38, tree_eng=nc.gpsimd)
            ga = emit_half(15, 0, 15)           # DVE half-tree
            # s4_14 join + clamps on Pool into s4c[:,0,:]
            s4_14 = pS.tile([128, 1, PX], bf16, tag="s4")
            eng_add(nc.gpsimd, s4_14[:, 0, :], ha[:, :], hb[:, :])
            emit_clamps(s4_14[:, :, :], s4c[:, 0:1, :], 1, clamp_eng=nc.vector)
            gb = emit_half(15, 1, 39)           # DVE half-tree
            # unit 15 join + clamps on DVE into s4c[:,1,:]
            s4_15 = pS.tile([128, 1, PX], bf16, tag="s4")
            nc.vector.tensor_tensor(
                out=s4_15[:, 0, :], in0=ga[:, :], in1=gb[:, :], op=Alu.add
            )
            emit_clamps(s4_15[:, :, :], s4c[:, 1:2, :], 1, clamp_eng=nc.vector)
            emit_group_macc(2)  # batch 2 masked accumulate (lagged)
            # group (12,13) masked-sum; emitted AFTER exp15b so it cannot
            # head-of-line block the last exp in Act's in-order queue
            emit_group_lse(3)  # units 12-13 -> Ln
            emit_group_macc(3)  # -> col 19
            # single ln-accum over clamped units 14+15 -> col 20
            lse_c = pS.tile([128, 2, PX], bf16, tag="lseg")
            nc.scalar.activation(
                lse_c[:, :, :], s4c[:, :, :], Ln, accum_out=accs[:, 20:21]
            )

            nc.sync.dma_start(acc_out[:, :], accs[:, :])

    nc.finalize()
    return nc


def _get_nc():
    with _lock:
        if "nc" not in _cache:
            _cache["nc"] = _build_nc()
        return _cache["nc"]


def kernel(reg_logits: np.ndarray, targets: np.ndarray, pos_mask: np.ndarray) -> np.ndarray:
    from concourse.bass_utils import run_bass_kernel_spmd

    nc = _get_nc()

    reg_logits = np.ascontiguousarray(reg_logits, dtype=np.float32).reshape(B, C, HW)
    targets = np.ascontiguousarray(targets, dtype=np.float32)
    mask_u8 = np.ascontiguousarray(pos_mask).astype(np.uint8)

    in_maps = []
    for c in range(NCORES):
        b0 = c * BPC
        in_maps.append(
            {
                "x": reg_logits[b0 : b0 + BPC],
                "tg": targets[b0 : b0 + BPC],
                "mk": mask_u8[b0 : b0 + BPC],
            }
        )

    res = run_bass_kernel_spmd(nc, in_maps, core_ids=list(range(NCORES)))

    tot_interp = 0.0
    tot_lse = 0.0
    npos = 0.0
    for r in res.results:
        a = r["acc"].astype(np.float64)
        tot_interp += a[:, :16].sum() + a[:, 36:40].sum()
        tot_lse += a[:, 16:21].sum()
        npos += a[:, 32:36].sum()

    total = tot_lse - tot_interp
    loss = total / (max(npos, 1.0) * 4.0) if npos > 0 else 0.0
    return np.float32(loss)


if __name__ == "__main__":
    rng = np.random.default_rng(0)
    rl = rng.standard_normal((B, C, H, W), dtype=np.float32)
    tg = (rng.random((B, HW, NJ), dtype=np.float32) * (BINS - 1)).astype(np.float32)
    pm = rng.integers(0, 2, size=(B, HW)).astype(bool)
    print(kernel(reg_logits=rl, targets=tg, pos_mask=pm))


# revision 64
# speedup vs baseline: 1.0679x; 1.0220x over previous
"""DFL loss (nn_DFLLoss) Trainium2 Bass kernel — 8-core data parallel.

reference computes, per (batch, pixel, coord j in 0..3):
    rl[b, hw, j, k] = reg_logits[b, j*8+k, hw]          (k in 0..7 bins)
    t = clip(targets, 0, 6.9999); l = floor(t); u = l+1
    per = w_l * (lse - rl[l]) + w_u * (lse - rl[u]),  lse = logsumexp_k rl
    loss = sum(per * pos_mask) / (max(sum(pos_mask), 1) * 4)

Key identity used here (removes the gather):
    w_l*rl[l] + w_u*rl[u] = sum_k relu(1 - |t - k|) * rl[k]
so masked_total = sum(mask*lse) - sum_k relu(1-|t-k|)*rl[k]*mask. The
hat-product+reduce runs as ONE fused custom DVE op per (batch, coord)
with the bin index k supplied by PageIdx over the 8 channel pages.
The mask is folded into t'' = clip(t) + 100*mask and the op evaluates
relu(1 - |t'' - (100 + k)|): positive pixels give |t - k|, masked-out
pixels give |t - 100 - k| >= 93 so every hat weight is exactly 0.

Engine budget (per core, 4 batches x 4 coords = 16 units):
  DMA    41.6us  <- the memory-roofline pole (14.85 MB f32 at 360 B/ns)
  DVE    ~39us   16 custom hat ops (1x only; custom DVE ops have no fast
                 modes) + first two bin-sum tree levels as 2x bf16
                 tensor_tensor + batch-0 prep during the DMA ramp
  Act    ~31us   exp + ln (no fast modes exist on Act)
  Pool   ~20us   prep batches 1-3, final tree level, masked-lse + npos
                 accumulations
Everything except the DMA stream and its ramp/tail overlaps under it.
"""

import threading
from operator import add as _operator_add

import numpy as np

BINS = 8
B, C, H, W = 32, 32, 160, 160
HW = H * W  # 25600
NCORES = 8
BPC = B // NCORES  # 4 batches per core
PX = HW // 128  # 200 pixels per partition per batch
NJ = 4

_lock = threading.Lock()
_cache: dict = {}


def _register_hat_op():
    """Register the fused hat*logit+reduce custom DVE op (idempotent)."""
    import concourse.dve_ops as dve_ops
    from concourse.dve_spec import (
        C0,
        C1,
        PageIdx,
        Spec,
        Src0,
        Src1,
        Zero,
        One,
        lower,
        maxx,
        relu,
    )
    from concourse.dve_uop import DveOpSpec

    name = "HAT_MUL_ACC_DFL"
    if name in dve_ops._SUB_OPCODE_FOR_NAME:
        for op in dve_ops.OPS:
            if op.name == name:
                return op

    _pg = PageIdx(C0, C1)  # idx = s0 + s1*page  (page = bin k)
    _d = Src0 - _pg

    def _ref(in0, in1, s0, s1, imm2):
        P, S, N = in0.shape
        idx = (s0 + s1 * np.arange(S)).reshape(1, S, 1)
        hat = np.maximum(1.0 - np.abs(in0.astype(np.float32) - idx), 0.0)
        body = (hat * in1).astype(np.float32)
        return body, body.reshape(P, -1).sum(-1, keepdims=True)

    spec = Spec(
        body=relu(One - maxx(_d, Zero - _d)) * Src1,
        accum=_operator_add,
        accum_init=Zero,
        reference=_ref,
    )
    shas = {}
    for ver in ("v3", "v4"):
        uops = lower(spec, ver=ver)
        shas[ver] = DveOpSpec(name=name, opcode=1, uops=uops, rd1_en=True).sha(ver)
    op = dve_ops.DveOp(name, spec, subdim=True, uops_sha=shas)
    row = dve_ops._CUSTOM_DVE_ROW_BASE + len(dve_ops.OPS)
    assert row < 0x20, "custom DVE opcode rows exhausted"
    dve_ops.OPS.append(op)
    dve_ops.CUSTOM_DVE_SPECS[name] = op.spec
    dve_ops._SUB_OPCODE_FOR_NAME[name] = row
    return op


def _patch_act_tables():
    """Force Exp and Ln to resolve to the one table set containing both.

    The act-table-load pass assigns each activation the first set containing
    its function; Exp->exp_and_others and Ln->natural_log would otherwise
    alternate table loads (~1.3us each) on every exp->ln transition. Removing
    the two functions from every other set (list order and ids preserved)
    makes natural_log_exp_and_others serve both: one load for the kernel.
    """
    import concourse.bacc as bacc
    import concourse.hw_specs as hw_specs
    import concourse.mybir as mybir

    if getattr(_patch_act_tables, "_done", False):
        return
    orig = hw_specs.get_activation_tables
    Exp = mybir.ActivationFunctionType.Exp
    Ln = mybir.ActivationFunctionType.Ln

    def patched(module_arch):
        t = orig(module_arch)
        both = t.get("natural_log_exp_and_others")
        if both is not None and Exp in both and Ln in both:
            for name, fns in t.items():
                if name != "natural_log_exp_and_others":
                    fns.discard(Exp)
                    fns.discard(Ln)
        return t

    hw_specs.get_activation_tables = patched
    bacc.get_activation_tables = patched
    _patch_act_tables._done = True


def _build_nc():
    import concourse.bacc as bacc
    import concourse.mybir as mybir
    from concourse.tile import TileContext

    _patch_act_tables()
    hat_op = _register_hat_op()
    f32 = mybir.dt.float32
    bf16 = mybir.dt.bfloat16
    u8 = mybir.dt.uint8

    nc = bacc.Bacc("TRN2", target_bir_lowering=False, debug=False)
    x = nc.dram_tensor("x", [BPC, C, HW], f32, kind="ExternalInput")
    tg = nc.dram_tensor("tg", [BPC, HW, NJ], f32, kind="ExternalInput")
    mk = nc.dram_tensor("mk", [BPC, HW], u8, kind="ExternalInput")
    # acc columns: [0:16] interp (unit u = b*4+j), [16:19] lse for batch
    # groups b0/b1/b2 (masked-sum via STT), [19] lse group (units 12,13),
    # [20] lse unit 14, [21] lse unit 15 (the latter three via the mask-blend
    # s4' = max(min(s4, m?3e38:1), 1-m) so ln(s4') accumulates pre-masked),
    # [32:36] npos per batch, [36:40] second-half interp for bin-split units
    acc_out = nc.dram_tensor("acc", [128, 40], f32, kind="ExternalOutput")

    # DRAM views (per batch): partition p = pixel-block of 200 px
    x_v = x.rearrange("b c (blk px) -> b blk c px", px=PX)  # [4,128,32,200]
    tg_v = tg.rearrange("b (blk pj) j -> b blk (pj j)", blk=128)  # [4,128,800]
    mk_v = mk.rearrange("b (blk px) -> b blk px", px=PX)  # [4,128,200]

    Exp = mybir.ActivationFunctionType.Exp
    Ln = mybir.ActivationFunctionType.Ln
    Alu = mybir.AluOpType

    with TileContext(nc) as tc:
        with (
            tc.tile_pool(name="pL", bufs=5) as pL,
            tc.tile_pool(name="pE", bufs=5) as pE,
            tc.tile_pool(name="pS", bufs=8) as pS,
            tc.tile_pool(name="pHat", bufs=3) as pHat,
            tc.tile_pool(name="pT", bufs=2) as pT,
            tc.tile_pool(name="pOnce", bufs=1) as pOnce,
        ):
            accs = pOnce.tile([128, 40], f32)
            nc.gpsimd.memset(accs[:, :], 0.0)  # unused cols must read as 0
            mf100s: dict = {}
            t2s: dict = {}
            tails: dict = {}
            s4gs: dict = {}

            def eng_add(eng, out, in0, in1):
                """Plain adds; scalar_tensor_tensor is NOT a legal Pool
                opcode in walrus codegen, so both engines use tensor_tensor
                (2x bf16 on DVE, Add-efficiency 0.42 on Pool)."""
                eng.tensor_tensor(out=out, in0=in0, in1=in1, op=Alu.add)

            def emit_batch_prep(b, tclp_eng=None):
                """mk/tg DMAs + mask/target prep. Emitted one unit ahead of
                the batch's first L so t'' is ready for its hats.
                Mask trick: t'' = clip(t) - 100*(1-m) so positive pixels
                keep t exactly (bf16-safe near 0..7) and masked pixels sit
                ~100 below every bin index (hat weight exactly 0; s0=0)."""
                t_raw = pT.tile([128, PX * NJ], f32, tag="t_raw")  # (px, j)
                m_raw = pT.tile([128, PX], u8, tag="m_raw")
                m1 = pT.tile([128, PX], bf16, tag="m1")
                mc100 = pT.tile([128, PX], bf16, tag="mc100")
                tclp = pT.tile([128, NJ, PX], bf16, tag="tclp")
                t2 = pT.tile([128, NJ, PX], bf16, tag="t2")
                mf100s[b] = m1
                t2s[b] = t2
                nc.sync.dma_start(m_raw[:, :], mk_v[b])
                nc.sync.dma_start(t_raw[:, :], tg_v[b])
                # m1 = mask as 1.0/0.0 (exact in bf16)
                nc.gpsimd.tensor_scalar(
                    out=m1[:, :],
                    in0=m_raw[:, :],
                    scalar1=1.0,
                    scalar2=None,
                    op0=Alu.mult,
                )
                # mc100 = -100*(1-m) = 100*m - 100 (0 or -100, both exact)
                nc.gpsimd.tensor_scalar(
                    out=mc100[:, :],
                    in0=m_raw[:, :],
                    scalar1=100.0,
                    scalar2=-100.0,
                    op0=Alu.mult,
                    op1=Alu.add,
                )
                # tclp = min(t, 6.9999), j-major, bf16
                t_raw_v = t_raw[:, :].rearrange("p (px j) -> p j px", j=NJ)
                (tclp_eng or nc.gpsimd).tensor_scalar(
                    out=tclp[:, :, :],
                    in0=t_raw_v,
                    scalar1=float(BINS - 1) - 1e-4,
                    scalar2=None,
                    op0=Alu.min,
                )
                # t'' = tclp + mc100 (DVE 2x bf16; adding 0 is exact)
                nc.vector.tensor_tensor(
                    out=t2[:, :, :],
                    in0=tclp[:, :, :],
                    in1=mc100[:, :].unsqueeze(1).broadcast_to((128, NJ, PX)),
                    op=Alu.add,
                )
                # npos accum: sum(m1) (DVE, 4x bf16)
                np_scr = pT.tile([128, PX], bf16, tag="np_scr")
                nc.vector.tensor_scalar(
                    out=np_scr[:, :],
                    in0=m1[:, :],
                    scalar1=1.0,
                    scalar2=0.0,
                    op0=Alu.mult,
                    op1=Alu.add,  # reduce op for accum_out
                    accum_out=accs[:, 32 + b : 33 + b],
                )
                # group s4 buffer for this batch's lse values
                nj = NJ if b < 3 else NJ - 2  # batch 3: 14/15 handled apart
                s4g = pS.tile([128, nj, PX], bf16, tag="s4g")
                s4gs[b] = s4g

            def emit_unit_head(u):
                """DMA + hat + exp for a full unit."""
                b, j = divmod(u, NJ)
                L = pL.tile([128, BINS, PX], f32, tag="L")
                nc.sync.dma_start(L[:, :, :], x_v[b, :, 8 * j : 8 * j + 8, :])
                # interp: acc[:, u] = sum_k relu(1-|t-k|) * L_k  (DVE, 1x)
                hat_scr = pHat.tile([128, BINS, PX], bf16, tag="hat")
                nc.vector._custom_dve(
                    hat_op,
                    out=hat_scr[:, :, :],
                    in0=t2s[b][:, j, :].unsqueeze(1).broadcast_to((128, BINS, PX)),
                    in1=L[:, :, :],
                    s0=0.0,
                    s1=1.0,
                    accum_out=accs[:, u : u + 1],
                )
                E = pE.tile([128, BINS, PX], bf16, tag="E")
                nc.scalar.activation(E[:, :, :], L[:, :, :], Exp)
                tails[u] = E

            def emit_unit_tree(u, eng, s4_eng=None):
                """Lagged tree levels; s4 lands in the batch group buffer.
                The final (cheap) level defaults to Pool to unload DVE."""
                E = tails[u]
                b, j = divmod(u, NJ)
                s16 = pS.tile([128, 4, PX], bf16, tag="s16")
                eng_add(eng, s16[:, :, :], E[:, 0::2, :], E[:, 1::2, :])
                s8 = pS.tile([128, 2, PX], bf16, tag="s8")
                eng_add(eng, s8[:, :, :], s16[:, 0::2, :], s16[:, 1::2, :])
                s4e = s4_eng or nc.gpsimd
                eng_add(s4e, s4gs[b][:, j, :], s8[:, 0, :], s8[:, 1, :])

            lses: dict = {}

            def emit_group_lse(b, nj=None):
                """One ln for a batch group (slot into the boundary hole)."""
                s4g = s4gs[b]
                nj = nj or s4g.shape[1]
                lse = pS.tile([128, nj, PX], bf16, tag="lseg")
                nc.scalar.activation(lse[:, :, :], s4g[:, 0:nj, :], Ln)
                lses[b] = lse

            def emit_group_macc(b, eng=None):
                """Masked lse accumulate: tensor product on `eng` (Pool
                multiply is a real Q7 kernel), accumulate on DVE 4x —
                Pool tensor_scalar with accum_out fails the engine check."""
                eng = eng or nc.vector
                lse = lses[b]
                nj = lse.shape[1]
                lsem = pS.tile([128, nj, PX], bf16, tag="lsem")
                eng.tensor_tensor(
                    out=lsem[:, :, :],
                    in0=lse[:, :, :],
                    in1=mf100s[b][:, :].unsqueeze(1).broadcast_to((128, nj, PX)),
                    op=Alu.mult,
                )
                lse_scr = pS.tile([128, nj, PX], bf16, tag="lse_scr")
                nc.vector.tensor_scalar(
                    out=lse_scr[:, :, :],
                    in0=lsem[:, :, :],
                    scalar1=1.0,
                    scalar2=0.0,
                    op0=Alu.mult,
                    op1=Alu.add,
                    accum_out=accs[:, 16 + b : 17 + b],
                )

            def emit_half(u, h, acc_col, tree_eng=None):
                """One bin-half (4 bins) of a split unit: DMA + hat + exp +
                its half-tree down to a [128, PX] partial sum (returned)."""
                b, j = divmod(u, NJ)
                c0 = 8 * j + 4 * h
                tree_eng = tree_eng or nc.vector
                Lh = pL.tile([128, 4, PX], f32, tag="Lh")
                nc.sync.dma_start(Lh[:, :, :], x_v[b, :, c0 : c0 + 4, :])
                hat_scr = pHat.tile([128, 4, PX], bf16, tag="hath")
                nc.vector._custom_dve(
                    hat_op,
                    out=hat_scr[:, :, :],
                    in0=t2s[b][:, j, :].unsqueeze(1).broadcast_to((128, 4, PX)),
                    in1=Lh[:, :, :],
                    s0=4.0 * h,
                    s1=1.0,
                    accum_out=accs[:, acc_col : acc_col + 1],
                )
                Eh = pE.tile([128, 4, PX], bf16, tag="Eh")
                nc.scalar.activation(Eh[:, :, :], Lh[:, :, :], Exp)
                s16h = pS.tile([128, 2, PX], bf16, tag="s16h")
                eng_add(tree_eng, s16h[:, :, :], Eh[:, 0::2, :], Eh[:, 1::2, :])
                s8h = pS.tile([128, PX], bf16, tag="s8h")
                eng_add(tree_eng, s8h[:, :], s16h[:, 0, :], s16h[:, 1, :])
                return s8h

            def emit_clamps(src, dst_slice, nj, clamp_eng):
                """Mask-blend s4' = max(min(s4, hi3), lo3) into dst_slice:
                positives keep s4, masked pixels become exactly 1 (ln -> 0)."""
                mn = pS.tile([128, nj, PX], bf16, tag="mn")
                hib = hi3[:, :].unsqueeze(1).broadcast_to((128, nj, PX))
                lob = lo3[:, :].unsqueeze(1).broadcast_to((128, nj, PX))
                clamp_eng.tensor_tensor(out=mn[:, :, :], in0=src, in1=hib, op=Alu.min)
                clamp_eng.tensor_tensor(out=dst_slice, in0=mn[:, :, :], in1=lob, op=Alu.max)

            # --- emission: software-pipelined over units -------------------
            emit_batch_prep(0, tclp_eng=nc.vector)
            emit_unit_head(0)

            # Trees of mid-batch units go to Pool; the j==3 unit's tree is
            # emitted un-lagged so the batch group's ln+macc slot into the
            # Act hole at the batch boundary (mk/tg DMAs delay the next L).
            POOL_TREES = {2, 6, 10, 12}
            for u in range(1, 14):
                b, j = divmod(u, NJ)
                if j == 2 and b < 3:
                    emit_batch_prep(b + 1)
                emit_unit_head(u)
                prev = u - 1
                if prev >= 0 and prev % NJ != 3:
                    eng = nc.gpsimd if prev in POOL_TREES else nc.vector
                    emit_unit_tree(prev, eng)
                if j == 3:
                    # un-lagged tree; s4 on Pool like every other group-tile
                    # writer (cross-engine slice writes serialize falsely)
                    emit_unit_tree(u, nc.vector)
                    if b < 2:
                        emit_group_lse(b)  # in the boundary hole
                elif j == 1 and 1 <= b <= 2:
                    emit_group_macc(b - 1)

            # batch-3 mask-blend clamp tensors (hi = m ? 3e38 : 1, lo = 1-m)
            hi3 = pOnce.tile([128, PX], bf16)
            nc.gpsimd.tensor_scalar(
                out=hi3[:, :],
                in0=mf100s[3][:, :],
                scalar1=3e38,
                scalar2=1.0,
                op0=Alu.mult,
                op1=Alu.add,
            )
            lo3 = pOnce.tile([128, PX], bf16)
            nc.gpsimd.tensor_scalar(
                out=lo3[:, :],
                in0=mf100s[3][:, :],
                scalar1=-1.0,
                scalar2=1.0,
                op0=Alu.mult,
                op1=Alu.add,
            )
            # clamped s4 buffer for units 14/15 -> one ln-accum (col 20)
            s4c = pS.tile([128, 2, PX], bf16, tag="s4c")

            # unit 14 full, unit 15 bin-halved. The three tail hats run
            # back-to-back on DVE (each half-head emits DMA+hat+exp only);
            # tree(13) goes to Pool so the DVE tail is just unit 14/15 work.
            emit_unit_head(14)

            def emit_half_head(h, acc_col):
                c0 = 8 * 3 + 4 * h
                Lh = pL.tile([128, 4, PX], f32, tag="Lh")
                nc.sync.dma_start(Lh[:, :, :], x_v[3, :, c0 : c0 + 4, :])
                hat_scr = pHat.tile([128, 4, PX], bf16, tag="hath")
                nc.vector._custom_dve(
                    hat_op,
                    out=hat_scr[:, :, :],
                    in0=t2s[3][:, 3, :].unsqueeze(1).broadcast_to((128, 4, PX)),
                    in1=Lh[:, :, :],
                    s0=4.0 * h,
                    s1=1.0,
                    accum_out=accs[:, acc_col : acc_col + 1],
                )
                Eh = pE.tile([128, 4, PX], bf16, tag="Eh")
                nc.scalar.activation(Eh[:, :, :], Lh[:, :, :], Exp)
                return Eh

            Ea = emit_half_head(0, 15)
            Eb = emit_half_head(1, 39)
            emit_unit_tree(13, nc.gpsimd)       # Pool; s4 feeds ln_g3
            # unit 14 tree fully on DVE into s4c[:,0,:]
            E14 = tails[14]
            s16_14 = pS.tile([128, 4, PX], bf16, tag="s16")
            eng_add(nc.vector, s16_14[:, :, :], E14[:, 0::2, :], E14[:, 1::2, :])
            s8_14 = pS.tile([128, 2, PX], bf16, tag="s8")
            eng_add(nc.vector, s8_14[:, :, :], s16_14[:, 0::2, :], s16_14[:, 1::2, :])
            s4_14 = pS.tile([128, 1, PX], bf16, tag="s4")
            eng_add(nc.vector, s4_14[:, 0, :], s8_14[:, 0, :], s8_14[:, 1, :])
            emit_clamps(s4_14[:, :, :], s4c[:, 0:1, :], 1, clamp_eng=nc.vector)
            # unit 15 half-trees + join + clamps on DVE into s4c[:,1,:]
            s16a = pS.tile([128, 2, PX], bf16, tag="s16h")
            eng_add(nc.vector, s16a[:, :, :], Ea[:, 0::2, :], Ea[:, 1::2, :])
            s8a = pS.tile([128, PX], bf16, tag="s8h")
            eng_add(nc.vector, s8a[:, :], s16a[:, 0, :], s16a[:, 1, :])
            s16b = pS.tile([128, 2, PX], bf16, tag="s16h")
            eng_add(nc.vector, s16b[:, :, :], Eb[:, 0::2, :], Eb[:, 1::2, :])
            s8b = pS.tile([128, PX], bf16, tag="s8h")
            eng_add(nc.vector, s8b[:, :], s16b[:, 0, :], s16b[:, 1, :])
            s4_15 = pS.tile([128, 1, PX], bf16, tag="s4")
            eng_add(nc.vector, s4_15[:, 0, :], s8a[:, :], s8b[:, :])
            emit_clamps(s4_15[:, :, :], s4c[:, 1:2, :], 1, clamp_eng=nc.vector)
            # group (12,13) masked-sum; emitted AFTER exp15b so it cannot
            # head-of-line block the last exp in Act's in-order queue
            emit_group_lse(2)  # batch 2 (deferred past the tail exps)
            emit_group_macc(2)
            emit_group_lse(3)  # units 12-13 -> Ln
            emit_group_macc(3, eng=nc.gpsimd)  # -> col 19
            # single ln-accum over clamped units 14+15 -> col 20
            lse_c = pS.tile([128, 2, PX], bf16, tag="lseg")
            nc.scalar.activation(
                lse_c[:, :, :], s4c[:, :, :], Ln, accum_out=accs[:, 20:21]
            )

            nc.sync.dma_start(acc_out[:, :], accs[:, :])

    nc.finalize()
    return nc


def _get_nc():
    with _lock:
        if "nc" not in _cache:
            _cache["nc"] = _build_nc()
        return _cache["nc"]


def kernel(reg_logits: np.ndarray, targets: np.ndarray, pos_mask: np.ndarray) -> np.ndarray:
    from concourse.bass_utils import run_bass_kernel_spmd

    nc = _get_nc()

    reg_logits = np.ascontiguousarray(reg_logits, dtype=np.float32).reshape(B, C, HW)
    targets = np.ascontiguousarray(targets, dtype=np.float32)
    mask_u8 = np.ascontiguousarray(pos_mask).astype(np.uint8)

    in_maps = []
    for c in range(NCORES):
        b0 = c * BPC
        in_maps.append(
            {
                "x": reg_logits[b0 : b0 + BPC],
                "tg": targets[b0 : b0 + BPC],
                "mk": mask_u8[b0 : b0 + BPC],
            }
        )

    res = run_bass_kernel_spmd(nc, in_maps, core_ids=list(range(NCORES)))

    tot_interp = 0.0
    tot_lse = 0.0
    npos = 0.0
    for r in res.results:
        a = r["acc"].astype(np.float64)
        tot_interp += a[:, :16].sum() + a[:, 36:40].sum()
        tot_lse += a[:, 16:21].sum()
        npos += a[:, 32:36].sum()

    total = tot_lse - tot_interp
    loss = total / (max(npos, 1.0) * 4.0) if npos > 0 else 0.0
    return np.float32(loss)


if __name__ == "__main__":
    rng = np.random.default_rng(0)
    rl = rng.standard_normal((B, C, H, W), dtype=np.float32)
    tg = (rng.random((B, HW, NJ), dtype=np.float32) * (BINS - 1)).astype(np.float32)
    pm = rng.integers(0, 2, size=(B, HW)).astype(bool)
    print(kernel(reg_logits=rl, targets=tg, pos_mask=pm))
